# revision 1
# baseline (speedup 1.0000x reference)
import sys
sys.path.insert(0, "/opt/trn_rl_repo")
import numpy as np
import ml_dtypes

import concourse.bass as bass
import concourse.bacc as bacc
import concourse.mybir as mybir
import concourse.tile as tile
from concourse.bass_utils import run_bass_kernel_spmd

fr = mybir.dt.float32r
f32 = mybir.dt.float32
bf = mybir.dt.bfloat16
AF = mybir.ActivationFunctionType
ALU = mybir.AluOpType

B, N, D = 2, 2048, 1024
H, DH = 16, 64
INNER, CTX, TV, FF = 1024, 256, 250, 4096
SCALE = DH ** -0.5
QC = 512
DT = 8
NCORES = 8

_CACHE = {}


def _run(gen):
    for _ in gen:
        pass


def _chain(*gens):
    for g in gens:
        yield from g


def _interleave(gp, ga):
    p_more = a_more = True
    while p_more or a_more:
        for _ in range(6):
            if p_more:
                p_more = next(gp, _DONE) is not _DONE
        if a_more:
            a_more = next(ga, _DONE) is not _DONE


_DONE = object()


def _build():
    nc = bacc.Bacc("TRN2", target_bir_lowering=False, debug=False,
                   enable_asserts=False, num_devices=NCORES)

    d_xT = nc.dram_tensor("xT", [D, N], fr, kind="ExternalInput").ap()
    d_ctxT = nc.dram_tensor("ctxT", [D, 256], fr, kind="ExternalInput").ap()
    d_hintT = nc.dram_tensor("hintT", [D, 256], fr, kind="ExternalInput").ap()
    d_w = {}
    for a in ("a1", "a2", "a3"):
        for wn, shp in (("wq", [D, INNER]), ("wk", [D, INNER]),
                        ("wv", [D, INNER]), ("wo", [INNER, D])):
            d_w[f"{a}_{wn}"] = nc.dram_tensor(f"{a}_{wn}", shp, fr,
                                              kind="ExternalInput").ap()
    d_w1 = nc.dram_tensor("ffw1", [D, 2 * FF], fr, kind="ExternalInput").ap()
    d_w2 = nc.dram_tensor("ffw2", [FF, D], fr, kind="ExternalInput").ap()
    d_lngb = nc.dram_tensor("lngb", [2, 4096], fr, kind="ExternalInput").ap()
    d_bias = nc.dram_tensor("biases", [128, 128], f32, kind="ExternalInput").ap()
    d_sel = nc.dram_tensor("sel", [128, 256], fr, kind="ExternalInput").ap()
    d_ones_fr = nc.dram_tensor("ones_fr", [128, 1], fr, kind="ExternalInput").ap()
    d_ones_bf = nc.dram_tensor("ones_bf", [128, 1], bf, kind="ExternalInput").ap()
    d_onesrow = nc.dram_tensor("onesrow", [1, QC], fr, kind="ExternalInput").ap()
    d_out = nc.dram_tensor("yT", [D, QC], f32, kind="ExternalOutput").ap()

    mm = nc.tensor.matmul
    stt = nc.vector.scalar_tensor_tensor
    tt = nc.vector.tensor_tensor

    with tile.TileContext(nc) as tc:
        from contextlib import ExitStack
        with ExitStack() as root:
            PP = root.enter_context(tc.tile_pool(name="PP", bufs=1,
                                                 space="PSUM"))
            p_const = root.enter_context(tc.tile_pool(name="const", bufs=1))
            p_xacc = root.enter_context(tc.tile_pool(name="xacc", bufs=1))
            p_w = root.enter_context(tc.tile_pool(name="w", bufs=10))
            p_sq = root.enter_context(tc.tile_pool(name="sq", bufs=2))
            p_small = root.enter_context(tc.tile_pool(name="small", bufs=1))
            p_gb = root.enter_context(tc.tile_pool(name="gb", bufs=2))
            p_t1 = root.enter_context(tc.tile_pool(name="t1", bufs=2))
            p_h = root.enter_context(tc.tile_pool(name="h", bufs=8))
            p_misc = root.enter_context(tc.tile_pool(name="misc", bufs=2))
            p_xin = root.enter_context(tc.tile_pool(name="xin", bufs=8))
            p_qt = root.enter_context(tc.tile_pool(name="qt", bufs=1))
            p_kt = root.enter_context(tc.tile_pool(name="kt", bufs=1))
            p_vt = root.enter_context(tc.tile_pool(name="vt", bufs=6))
            p_pt = root.enter_context(tc.tile_pool(name="pt", bufs=5))
            p_osb = root.enter_context(tc.tile_pool(name="osb", bufs=1))
            p_dsb = root.enter_context(tc.tile_pool(name="dsb", bufs=1))
            p_kvs = root.enter_context(tc.tile_pool(name="kvs", bufs=8))

            def PS(bank, T=QC, name="ps"):
                return PP.tile([128, T], f32, tag=f"b{bank}",
                               name=f"{name}b{bank}")

            biases = p_const.tile([128, 128], f32)
            nc.sync.dma_start(biases[:], d_bias[:])
            sel = p_const.tile([128, 256], fr)
            nc.sync.dma_start(sel[:], d_sel[:])
            ones_fr = p_const.tile([128, 1], fr)
            nc.sync.dma_start(ones_fr[:], d_ones_fr[:])
            ones_bf = p_const.tile([128, 1], bf)
            nc.sync.dma_start(ones_bf[:], d_ones_bf[:])
            epsc = p_const.tile([1, 1], f32)
            nc.vector.memset(epsc[:], 1e-5)

            xacc = []
            for m in range(DT):
                xa = p_xacc.tile([128, QC], fr, name=f"xacc{m}", tag=f"xacc{m}")
                nc.sync.dma_start(xa[:], d_xT[m * 128:(m + 1) * 128, 0:QC])
                xacc.append(xa)

            def gen_ln(src, ln_idx, out):
                s1 = PS(0, name="s1")
                s2 = PS(1, name="s2")
                for d in range(DT):
                    sq = p_sq.tile([128, QC], fr, tag="sq", name="sq")
                    nc.scalar.activation(sq[:], src[d][:], AF.Square)
                    mm(s1[:1, :], ones_fr[:], src[d][:],
                       start=(d == 0), stop=(d == DT - 1))
                    mm(s2[:1, :], ones_fr[:], sq[:],
                       start=(d == 0), stop=(d == DT - 1))
                    if d % 2 == 1:
                        yield
                mu = p_small.tile([1, QC], f32, tag="mu", name="mu")
                nc.scalar.mul(mu[:], s1[:1, :], 1.0 / D)
                musq = p_small.tile([1, QC], f32, tag="musq", name="musq")
                nc.scalar.square(musq[:], mu[:])
                var = p_small.tile([1, QC], f32, tag="var", name="var")
                stt(var[:], s2[:1, :], 1.0 / D, musq[:], ALU.mult, ALU.subtract)
                nc.scalar.activation(var[:], var[:], AF.Sqrt,
                                     bias=epsc[0:1, 0:1])
                istd = p_small.tile([1, QC], fr, tag="istd", name="istd")
                with nc.allow_low_precision(reason="istd feeds fp32r matmul"):
                    nc.vector.reciprocal(istd[:], var[:])
                ab2 = p_small.tile([2, QC], fr, tag="ab2", name="ab2")
                nc.sync.dma_start(ab2[1:2, :], d_onesrow[:])
                stt(ab2[0:1, :], mu[:], -1.0, istd[:], ALU.mult, ALU.mult)
                yield
                for d in range(DT):
                    col = ln_idx * 1024 + d * 128
                    gb = p_gb.tile([2, 128], fr, tag="gb", name="gb")
                    nc.sync.dma_start(gb[:], d_lngb[0:2, col:col + 128])
                    a_ps = PS(2, name="abg")
                    mm(a_ps[:], gb[0:1, :], istd[:], start=True, stop=True)
                    b_ps = PS(3, name="bbg")
                    mm(b_ps[:], gb[0:2, :], ab2[:], start=True, stop=True)
                    t1 = p_t1.tile([128, QC], f32, tag="t1", name="t1")
                    tt(t1[:], src[d][:], a_ps[:], ALU.mult)
                    ht = p_h.tile([128, QC], fr, tag="h1", name="ht")
                    tt(ht[:], t1[:], b_ps[:], ALU.add)
                    out.append(ht)
                    if d % 2 == 1:
                        yield

            def load_w(dram, k, half):
                wt = p_w.tile([128, 512], fr, tag="w", name="wt")
                nc.sync.dma_start(
                    wt[:], dram[k * 128:(k + 1) * 128,
                                half * 512:(half + 1) * 512])
                return wt

            def gen_project(wdram, rhs, T, out_tag, out_pool, out):
                for p in range(2):
                    pss = [PS(j, T, name=f"pj{p}{j}") for j in range(4)]
                    for k in range(DT):
                        wt = load_w(wdram, k, p)
                        for j in range(4):
                            mm(pss[j][:, 0:T], wt[:, j * 128:(j + 1) * 128],
                               rhs[k][:], start=(k == 0), stop=(k == DT - 1))
                        yield
                    for j in range(4):
                        m = p * 4 + j
                        ot = out_pool.tile([128, T], fr, tag=f"{out_tag}{m}",
                                           name=f"{out_tag}{m}")
                        nc.vector.tensor_copy(ot[:], pss[j][:, 0:T])
                        out.append(ot)
                    yield

            def gen_vproj(wdram, src, nsub, out):
                vts = [p_vt.tile([128, 1024], bf, tag="vt", name="vt")
                       for _ in range(nsub)]
                for half in range(2):
                    pss = [PS(j, name=f"v{half}{j}") for j in range(nsub)]
                    for k in range(DT):
                        wt = load_w(wdram, k, half)
                        for s in range(nsub):
                            mm(pss[s][:], src[k][:, s * 128:(s + 1) * 128],
                               wt[:], start=(k == 0), stop=(k == DT - 1))
                        yield
                    for s in range(nsub):
                        nc.vector.tensor_copy(
                            vts[s][:, half * 512:(half + 1) * 512], pss[s][:])
                    yield
                out.extend(vts)

            def gen_kvload(dram, out):
                for d in range(DT):
                    kv = p_kvs.tile([128, 256], fr, tag="kvs", name="kv")
                    nc.sync.dma_start(kv[:], dram[d * 128:(d + 1) * 128, :])
                    out.append(kv)

            def gen_att(KT, Vt, QT, nsub, osb, dsb, accumulate, dfix):
                sflip = 0
                dq = None
                for pair in range(8):
                    q = pair // 2
                    if pair % 2 == 0:
                        dq = PS(7, name="dq")
                    o_ps = PS(6, name="ops")
                    for sub in range(nsub):
                        for hh in range(2):
                            h_ = 2 * pair + hh
                            s_ps = PS(4 + sflip, name="sps")
                            sflip ^= 1
                            mm(s_ps[:],
                               KT[pair][hh * 64:(hh + 1) * 64,
                                        sub * 128:(sub + 1) * 128],
                               QT[pair][hh * 64:(hh + 1) * 64, :],
                               start=True, stop=True)
                            pt = p_pt.tile([128, QC], bf, tag="pt", name="pt")
                            nc.scalar.activation(pt[:], s_ps[:], AF.Exp,
                                                 scale=SCALE)
                            mm(o_ps[hh * 64:(hh + 1) * 64, :],
                               Vt[sub][:, h_ * 64:(h_ + 1) * 64], pt[:],
                               start=(sub == 0), stop=(sub == nsub - 1),
                               tile_position=(0, hh * 64),
                               skip_group_check=True)
                            pos = 32 * (h_ % 4)
                            mm(dq[pos:pos + 1, :], ones_bf[:], pt[:],
                               start=(sub == 0), stop=(sub == nsub - 1),
                               tile_position=(0, pos), skip_group_check=True)
                    if accumulate:
                        tt(osb[pair][:], o_ps[:], osb[pair][:], ALU.add)
                    else:
                        nc.vector.tensor_copy(osb[pair][:], o_ps[:])
                    if pair % 2 == 1:
                        if accumulate:
                            tt(dsb[q][:], dq[:], dsb[q][:], ALU.add)
                        elif dfix != 0.0:
                            nc.vector.tensor_scalar_add(dsb[q][:], dq[:], dfix)
                        else:
                            nc.vector.tensor_copy(dsb[q][:], dq[:])
                    yield

            def normalize(osb, dsb):
                for pair in range(8):
                    bc = PS(4 + (pair % 2), name="bc")
                    mm(bc[:], sel[:, (pair % 2) * 128:(pair % 2 + 1) * 128],
                       dsb[pair // 2][:], start=True, stop=True)
                    rc = p_misc.tile([128, QC], f32, tag="rc", name="rc")
                    nc.vector.reciprocal(rc[:], bc[:])
                    tt(osb[pair][:], osb[pair][:], rc[:], ALU.mult)

            def outproj(wdram, osb, bias_col):
                yps = [PS(m, name=f"y{m}") for m in range(8)]
                for k in range(DT):
                    wha = load_w(wdram, k, 0)
                    whb = load_w(wdram, k, 1)
                    for m in range(DT):
                        wt = wha if m < 4 else whb
                        mm(yps[m][:], wt[:, (m % 4) * 128:(m % 4 + 1) * 128],
                           osb[k][:], start=(k == 0), stop=(k == DT - 1))
                for m in range(DT):
                    stt(xacc[m][:], yps[m][:],
                        biases[:, bias_col + m:bias_col + m + 1],
                        xacc[m][:], ALU.add, ALU.add)

            def alloc_osb(pfx):
                o = [p_osb.tile([128, QC], fr, name=f"{pfx}o{m}",
                                tag=f"osb{m}") for m in range(DT)]
                d = [p_dsb.tile([128, QC], fr, name=f"{pfx}d{q}",
                                tag=f"dsb{q}") for q in range(4)]
                return o, d

            osb, dsb = alloc_osb("s")
            QT, att_prev = [], None
            KTs, Vts = {}, {}
            for kc in range(4):
                if kc == 0:
                    src = xacc
                else:
                    src = []
                    for d in range(DT):
                        xt = p_xin.tile([128, QC], fr, tag="xin", name="xt")
                        nc.sync.dma_start(
                            xt[:], d_xT[d * 128:(d + 1) * 128,
                                        kc * QC:(kc + 1) * QC])
                        src.append(xt)
                h1 = []
                KTs[kc], Vts[kc] = [], []
                parts = [gen_ln(src, 0, h1)]
                if kc == 0:
                    parts.append(gen_project(d_w["a1_wq"], h1, QC, "qt",
                                             p_qt, QT))
                parts.append(gen_project(d_w["a1_wk"], h1, QC, "kt",
                                         p_kt, KTs[kc]))
                parts.append(gen_vproj(d_w["a1_wv"], h1, 4, Vts[kc]))
                gp = _chain(*parts)
                if att_prev is None:
                    _run(gp)
                else:
                    _interleave(gp, att_prev)
                att_prev = gen_att(KTs[kc], Vts[kc], QT, 4, osb, dsb,
                                   accumulate=(kc != 0), dfix=0.0)
            kvs2, KT2, Vt2 = [], [], []
            gen_kvload(d_ctxT, kvs2)
            ctx_prep = _chain(gen_project(d_w["a2_wk"], kvs2, 256, "kt",
                                          p_kt, KT2),
                              gen_vproj(d_w["a2_wv"], kvs2, 2, Vt2))
            _interleave(ctx_prep, att_prev)
            normalize(osb, dsb)
            outproj(d_w["a1_wo"], osb, 0)

            h2, QT2 = [], []
            _run(_chain(gen_ln(xacc, 1, h2),
                        gen_project(d_w["a2_wq"], h2, QC, "qt", p_qt, QT2)))
            osb, dsb = alloc_osb("c")
            kvs3, KT3, Vt3 = [], [], []
            gen_kvload(d_hintT, kvs3)
            hint_prep = _chain(gen_project(d_w["a3_wk"], kvs3, 256, "kt",
                                           p_kt, KT3),
                               gen_vproj(d_w["a3_wv"], kvs3, 2, Vt3))
            _interleave(hint_prep,
                        gen_att(KT2, Vt2, QT2, 2, osb, dsb, False, 0.0))
            normalize(osb, dsb)
            outproj(d_w["a2_wo"], osb, 8)

            h3, QT3 = [], []
            _run(_chain(gen_ln(xacc, 2, h3),
                        gen_project(d_w["a3_wq"], h3, QC, "qt", p_qt, QT3)))
            osb, dsb = alloc_osb("h")
            _run(gen_att(KT3, Vt3, QT3, 2, osb, dsb, False, float(TV - 256)))
            normalize(osb, dsb)
            outproj(d_w["a3_wo"], osb, 16)

            h4 = []
            _run(gen_ln(xacc, 3, h4))
            ut = []

            def u_slot(i):
                if i < 8:
                    return p_xin.tile([128, QC], fr, tag="xin", name=f"u{i}")
                if i < 16:
                    return p_kt.tile([128, QC], fr, tag=f"kt{i-8}",
                                     name=f"u{i}")
                if i < 24:
                    return p_qt.tile([128, QC], fr, tag=f"qt{i-16}",
                                     name=f"u{i}")
                return p_osb.tile([128, QC], fr, tag=f"osb{i-24}",
                                  name=f"u{i}")

            for fc in range(8):
                aps = [PS(j, name=f"fa{j}") for j in range(4)]
                gps = [PS(4 + j, name=f"fg{j}") for j in range(4)]
                for k in range(DT):
                    wa = p_w.tile([128, 512], fr, tag="w", name="wa")
                    nc.sync.dma_start(
                        wa[:], d_w1[k * 128:(k + 1) * 128,
                                    fc * 512:(fc + 1) * 512])
                    wg = p_w.tile([128, 512], fr, tag="w", name="wg")
                    nc.sync.dma_start(
                        wg[:], d_w1[k * 128:(k + 1) * 128,
                                    FF + fc * 512:FF + (fc + 1) * 512])
                    for j in range(4):
                        mm(aps[j][:], wa[:, j * 128:(j + 1) * 128], h4[k][:],
                           start=(k == 0), stop=(k == DT - 1))
                        mm(gps[j][:], wg[:, j * 128:(j + 1) * 128], h4[k][:],
                           start=(k == 0), stop=(k == DT - 1))
                for j in range(4):
                    blk = fc * 4 + j
                    gl = p_misc.tile([128, QC], f32, tag="gl", name="gl")
                    nc.scalar.activation(gl[:], gps[j][:], AF.Gelu,
                                         bias=biases[:, 64 + blk:65 + blk])
                    u = u_slot(blk)
                    stt(u[:], aps[j][:], biases[:, 32 + blk:33 + blk], gl[:],
                        ALU.add, ALU.mult)
                    ut.append(u)
            yps = [PS(m, name=f"y2{m}") for m in range(8)]
            for kk in range(32):
                wha = load_w(d_w2, kk, 0)
                whb = load_w(d_w2, kk, 1)
                for m in range(DT):
                    wt = wha if m < 4 else whb
                    mm(yps[m][:], wt[:, (m % 4) * 128:(m % 4 + 1) * 128],
                       ut[kk][:], start=(kk == 0), stop=(kk == 31))
            for m in range(DT):
                stt(xacc[m][:], yps[m][:], biases[:, 24 + m:25 + m],
                    xacc[m][:], ALU.add, ALU.add)

            for m in range(DT):
                nc.sync.dma_start(d_out[m * 128:(m + 1) * 128, :],
                                  xacc[m][:].bitcast(f32))

    nc.compile()
    return nc


def _sin_pe(T, d):
    pos = np.arange(T, dtype=np.float32)[:, None]
    den = np.power(10000.0, 2.0 * np.arange(d // 2, dtype=np.float32) / d
                   ).astype(np.float32)
    ang = pos / den
    return np.stack([np.sin(ang), np.cos(ang)], -1).reshape(T, d
                                                            ).astype(np.float32)


def _pack_bias(v, n):
    return np.ascontiguousarray(np.asarray(v, np.float32).reshape(n, 128).T)


def kernel(**inputs):
    if "nc" not in _CACHE:
        _CACHE["nc"] = _build()
    nc = _CACHE["nc"]

    f = lambda k: np.ascontiguousarray(np.asarray(inputs[k], np.float32))
    x = f("x")
    ctx = f("context")
    hint = f("hint_control") + _sin_pe(TV, D)[None]

    shared = {}
    for a in ("a1", "a2", "a3"):
        for wn in ("wq", "wk", "wv", "wo"):
            shared[f"{a}_{wn}"] = f(f"{a}_{wn}")
    shared["ffw1"] = f("ff_w1")
    shared["ffw2"] = f("ff_w2")
    lngb = np.zeros((2, 4096), np.float32)
    for i, ln in enumerate(("ln1", "ln2", "ln4", "ln3")):
        lngb[0, i * 1024:(i + 1) * 1024] = f(f"{ln}_g")
        lngb[1, i * 1024:(i + 1) * 1024] = f(f"{ln}_b")
    shared["lngb"] = lngb
    bias = np.zeros((128, 128), np.float32)
    bias[:, 0:8] = _pack_bias(inputs["a1_bo"], 8)
    bias[:, 8:16] = _pack_bias(inputs["a2_bo"], 8)
    bias[:, 16:24] = _pack_bias(inputs["a3_bo"], 8)
    bias[:, 24:32] = _pack_bias(inputs["ff_b2"], 8)
    bias[:, 32:96] = _pack_bias(inputs["ff_b1"], 64)
    shared["biases"] = bias
    sel = np.zeros((128, 256), np.float32)
    sel[0, 0:64] = 1.0
    sel[32, 64:128] = 1.0
    sel[64, 128:192] = 1.0
    sel[96, 192:256] = 1.0
    shared["sel"] = sel
    shared["ones_fr"] = np.ones((128, 1), np.float32)
    shared["ones_bf"] = np.ones((128, 1), ml_dtypes.bfloat16)
    shared["onesrow"] = np.ones((1, QC), np.float32)

    in_maps = []
    for c in range(NCORES):
        b, r = c // 4, c % 4
        order = [r] + [j for j in range(4) if j != r]
        xperm = np.concatenate([x[b, j * QC:(j + 1) * QC] for j in order], 0)
        m = dict(shared)
        m["xT"] = np.ascontiguousarray(xperm.T)
        m["ctxT"] = np.ascontiguousarray(ctx[b].T)
        hT = np.zeros((D, 256), np.float32)
        hT[:, :TV] = hint[b].T
        m["hintT"] = hT
        in_maps.append(m)

    _CACHE["in_maps"] = in_maps
    res = run_bass_kernel_spmd(nc, in_maps, core_ids=list(range(NCORES)))
    out = np.zeros((B, N, D), np.float32)
    for c in range(NCORES):
        b, r = c // 4, c % 4
        out[b, r * QC:(r + 1) * QC] = res.results[c]["yT"].T
    return out

